# revision 8
# baseline (speedup 1.0000x reference)
"""Trainium2 Bass kernel for nn_Net_17179869915 (binarized dense MLP).

Network (reference semantics, B = 32768):
    h1 = x @ sign(w1).T + b1                      # [B, 64]
    s  = sign(h1 - mean(h1))                      # bn1 scale/clip are sign-invariant
    h2 = s @ sign(w2).T                           # b2 cancels inside bn2
    z  = clip((h2 - mean(h2)) * rsqrt(var(h2) + 1e-5), -1, 1)
    out = z @ w3.T + b3                           # [B, 10]

Data-parallel over 8 NeuronCores (4096 rows each); BN statistics are exact
(global) via two tiny AllReduces.

fc1 precision trick: fp32 matmul on the PE costs 4 cycles/row, but fp32r
(E8M11) runs at 1 cycle/row for free dim >= 256. We transpose x on the PE in
fp32, round to fp32r (DVE copy), compute the bf16 residual x - fp32r(x), and
accumulate  x@W = fp32r(x)@W + residual@W  in PSUM. Residual magnitude is
~2^-12 of x, stored in bf16, so the combined error is ~2^-21 per element —
below fp32 accumulation noise. Validated on HW: 1.6e-7 relative.
"""

import numpy as np
import ml_dtypes

import concourse.bass as bass
import concourse.tile as tile
from concourse import bacc, mybir

f32 = mybir.dt.float32
f32r = mybir.dt.float32r
bf16 = mybir.dt.bfloat16

B_TOTAL = 32768
N_CORES = 8
B_CORE = B_TOTAL // N_CORES      # 4096
BT = 512                         # batch tile (free dim of fc1 matmuls)
NJ = B_CORE // BT                # 8 batch tiles per core
NI = BT // 128                   # 4 natural x sub-tiles per batch tile
D_IN = 784
NK = 7                           # ceil(784 / 128) feature chunks
K_LAST = D_IN - 6 * 128          # 16
H = 64
D_OUT = 10
BN_EPS = 1e-5


def build(warmup=True, act_cast=True, ttr=False):
    nc = bacc.Bacc("TRN2", target_bir_lowering=False)

    x_d = nc.dram_tensor("x", [B_CORE, D_IN], f32, kind="ExternalInput")
    w1r_d = nc.dram_tensor("w1r", [NK * 128, H], f32r, kind="ExternalInput")
    w1b_d = nc.dram_tensor("w1b", [NK * 128, H], bf16, kind="ExternalInput")
    w2s_d = nc.dram_tensor("w2s", [H, H], bf16, kind="ExternalInput")
    w3t_d = nc.dram_tensor("w3t", [H, D_OUT], f32, kind="ExternalInput")
    b3_d = nc.dram_tensor("b3", [1, D_OUT], f32, kind="ExternalInput")
    eye_d = nc.dram_tensor("eye", [128, 128], f32, kind="ExternalInput")
    out_d = nc.dram_tensor("out", [B_CORE, D_OUT], f32, kind="ExternalOutput")

    with tile.TileContext(nc) as tc:
        with (
            tc.tile_pool(name="wpool", bufs=1) as wpool,
            tc.tile_pool(name="xin", bufs=2) as xin_pool,
            tc.tile_pool(name="xsplit", bufs=2) as xsplit_pool,
            tc.tile_pool(name="persist", bufs=1) as persist,
            tc.tile_pool(name="small", bufs=1) as small,
            tc.tile_pool(name="psum_xt", bufs=2, space="PSUM") as psum_xt,
            tc.tile_pool(name="psum_h", bufs=2, space="PSUM") as psum_h,
            tc.tile_pool(name="psum_o", bufs=2, space="PSUM") as psum_o,
            tc.tile_pool(name="dram", bufs=1, space="DRAM") as dram,
        ):
            # ---- weights / constants ----
            w1r_t = wpool.tile([128, NK, H], f32r)
            w1b_t = wpool.tile([128, NK, H], bf16)
            w2s_t = wpool.tile([H, H], bf16)
            w3t_t = wpool.tile([H, D_OUT], f32)
            eye_t = wpool.tile([128, 128], f32)
            b3row = wpool.tile([1, D_OUT], f32)
            b3bc = wpool.tile([128, D_OUT], f32)
            nc.sync.dma_start(w1r_t[:], w1r_d.ap().rearrange("(c p) h -> p c h", p=128))
            nc.sync.dma_start(w1b_t[:], w1b_d.ap().rearrange("(c p) h -> p c h", p=128))
            nc.sync.dma_start(w2s_t[:], w2s_d[:])
            nc.sync.dma_start(w3t_t[:], w3t_d[:])
            nc.sync.dma_start(b3row[:], b3_d[:])
            nc.sync.dma_start(eye_t[:], eye_d[:])
            nc.gpsimd.partition_broadcast(b3bc[:], b3row[:])

            # Warm-up collective: absorbs CC-core cold start and the
            # per-core launch stagger while phase A computes. Result unused.
            if warmup:
              wu_in = dram.tile([8], f32)
              wu_out = dram.tile([8], f32, addr_space="Shared")
              wu_sb = small.tile([1, 8], f32)
              nc.vector.memset(wu_sb[:], 0.0)
              nc.sync.dma_start(wu_in[:], wu_sb[:])
              nc.gpsimd.collective_compute(
                  "AllReduce",
                  mybir.AluOpType.add,
                  replica_groups=[list(range(N_CORES))],
                  ins=[wu_in.opt()],
                  outs=[wu_out.opt()],
              )
              wu_back = small.tile([1, 8], f32)
              nc.sync.dma_start(wu_back[:], wu_out[:])

            # ---- persistent activations (feature-major) ----
            h1T = persist.tile([H, B_CORE], f32)
            sT = persist.tile([H, B_CORE], bf16)
            h2T = persist.tile([H, B_CORE], f32)
            zT = persist.tile([H, B_CORE], f32)
            out_sb = persist.tile([128, B_CORE // 128, D_OUT], f32)

            h1sum = small.tile([H, NJ], f32)
            h2sum = small.tile([H, NJ], f32)
            h2ss = small.tile([H, NJ], f32)
            sq_scrap = small.tile([H, BT], f32)

            # ---- phase A: transpose x, split fp32r+residual, fc1 ----
            for j in range(NJ):
                x_nat = xin_pool.tile([128, NI, D_IN], f32)
                nc.sync.dma_start(
                    x_nat[:],
                    x_d.ap()[j * BT : (j + 1) * BT, :].rearrange(
                        "(i p) f -> p i f", p=128
                    ),
                )
                xr_t = xsplit_pool.tile([128, NK, BT], f32r, tag="xr")
                xres_t = xsplit_pool.tile([128, NK, BT], bf16, tag="xres")
                for k in range(NK):
                    kp = K_LAST if k == NK - 1 else 128
                    xt_psum = psum_xt.tile([128, BT], f32, tag="xt")
                    for i in range(NI):
                        nc.tensor.transpose(
                            xt_psum[0:kp, i * 128 : (i + 1) * 128],
                            x_nat[:, i, k * 128 : k * 128 + kp],
                            eye_t[:],
                        )
                    if act_cast:
                        nc.scalar.activation(
                            xr_t[0:kp, k, :], xt_psum[0:kp, :],
                            mybir.ActivationFunctionType.Copy,
                        )
                    else:
                        nc.vector.tensor_copy(xr_t[0:kp, k, :], xt_psum[0:kp, :])
                    nc.vector.tensor_tensor(
                        out=xres_t[0:kp, k, :],
                        in0=xt_psum[0:kp, :],
                        in1=xr_t[0:kp, k, :].bitcast(f32),
                        op=mybir.AluOpType.subtract,
                    )
                h1_psum = psum_h.tile([H, BT], f32, tag="h")
                for k in range(NK):
                    kp = K_LAST if k == NK - 1 else 128
                    nc.tensor.matmul(
                        h1_psum[:],
                        w1r_t[0:kp, k, :],
                        xr_t[0:kp, k, :],
                        start=(k == 0),
                        stop=False,
                    )
                for k in range(NK):
                    kp = K_LAST if k == NK - 1 else 128
                    nc.tensor.matmul(
                        h1_psum[:],
                        w1b_t[0:kp, k, :],
                        xres_t[0:kp, k, :],
                        start=False,
                        stop=(k == NK - 1),
                    )
                nc.scalar.activation(
                    h1T[:, j * BT : (j + 1) * BT],
                    h1_psum[:],
                    mybir.ActivationFunctionType.Copy,
                    accum_out=h1sum[:, j : j + 1],
                )

            # ---- phase B: global mean of h1 ----
            hsumL = small.tile([H, 1], f32)
            nc.vector.tensor_reduce(
                hsumL[:], h1sum[:], mybir.AxisListType.X, mybir.AluOpType.add
            )
            cc1_in = dram.tile([H], f32)
            cc1_out = dram.tile([H], f32, addr_space="Shared")
            nc.sync.dma_start(cc1_in[:], hsumL[:])
            nc.gpsimd.collective_compute(
                "AllReduce",
                mybir.AluOpType.add,
                replica_groups=[list(range(N_CORES))],
                ins=[cc1_in.opt()],
                outs=[cc1_out.opt()],
            )
            hsumG = small.tile([H, 1], f32)
            nc.sync.dma_start(hsumG[:], cc1_out[:])
            negmu1 = small.tile([H, 1], f32)
            nc.vector.tensor_scalar(
                out=negmu1[:], in0=hsumG[:], scalar1=-1.0 / B_TOTAL, scalar2=None,
                op0=mybir.AluOpType.mult,
            )

            # ---- phase C: sign, fc2, h2 stats ----
            for j in range(NJ):
                jsl = slice(j * BT, (j + 1) * BT)
                nc.scalar.activation(
                    sT[:, jsl], h1T[:, jsl],
                    mybir.ActivationFunctionType.Sign, bias=negmu1[:],
                )
                h2_psum = psum_h.tile([H, BT], f32, tag="h")
                nc.tensor.matmul(
                    h2_psum[:], w2s_t[:], sT[:, jsl], start=True, stop=True
                )
                nc.scalar.activation(
                    h2T[:, jsl], h2_psum[:],
                    mybir.ActivationFunctionType.Copy,
                    accum_out=h2sum[:, j : j + 1],
                )
                if ttr:
                    nc.vector.tensor_tensor_reduce(
                        out=sq_scrap[:],
                        in0=h2T[:, jsl],
                        in1=h2T[:, jsl],
                        scale=1.0,
                        scalar=0.0,
                        op0=mybir.AluOpType.mult,
                        op1=mybir.AluOpType.add,
                        accum_out=h2ss[:, j : j + 1],
                    )
                else:
                    nc.scalar.activation(
                        sq_scrap[:], h2_psum[:],
                        mybir.ActivationFunctionType.Square,
                        accum_out=h2ss[:, j : j + 1],
                    )

            # ---- phase D: global bn2 stats ----
            s2L = small.tile([H, 1], f32)
            ssL = small.tile([H, 1], f32)
            nc.vector.tensor_reduce(
                s2L[:], h2sum[:], mybir.AxisListType.X, mybir.AluOpType.add
            )
            nc.vector.tensor_reduce(
                ssL[:], h2ss[:], mybir.AxisListType.X, mybir.AluOpType.add
            )
            cc2_in = dram.tile([2 * H], f32)
            cc2_out = dram.tile([2 * H], f32, addr_space="Shared")
            nc.sync.dma_start(cc2_in[0:H], s2L[:])
            nc.sync.dma_start(cc2_in[H : 2 * H], ssL[:])
            nc.gpsimd.collective_compute(
                "AllReduce",
                mybir.AluOpType.add,
                replica_groups=[list(range(N_CORES))],
                ins=[cc2_in.opt()],
                outs=[cc2_out.opt()],
            )
            s2G = small.tile([H, 1], f32)
            ssG = small.tile([H, 1], f32)
            nc.sync.dma_start(s2G[:], cc2_out[0:H])
            nc.sync.dma_start(ssG[:], cc2_out[H : 2 * H])

            mu2 = small.tile([H, 1], f32)
            e2 = small.tile([H, 1], f32)
            mu2sq = small.tile([H, 1], f32)
            vareps = small.tile([H, 1], f32)
            rec = small.tile([H, 1], f32)
            inv2 = small.tile([H, 1], f32)
            nc.vector.tensor_scalar(
                out=mu2[:], in0=s2G[:], scalar1=1.0 / B_TOTAL, scalar2=None,
                op0=mybir.AluOpType.mult,
            )
            nc.vector.tensor_scalar(
                out=e2[:], in0=ssG[:], scalar1=1.0 / B_TOTAL, scalar2=None,
                op0=mybir.AluOpType.mult,
            )
            nc.vector.tensor_tensor(
                out=mu2sq[:], in0=mu2[:], in1=mu2[:], op=mybir.AluOpType.mult
            )
            nc.vector.tensor_tensor(
                out=vareps[:], in0=e2[:], in1=mu2sq[:], op=mybir.AluOpType.subtract
            )
            nc.vector.tensor_scalar(
                out=vareps[:], in0=vareps[:], scalar1=BN_EPS, scalar2=None,
                op0=mybir.AluOpType.add,
            )
            nc.vector.reciprocal(rec[:], vareps[:])
            nc.scalar.activation(
                inv2[:], rec[:], mybir.ActivationFunctionType.Sqrt
            )

            # ---- phase E: z = clip((h2 - mu2) * inv2) ----
            for j in range(NJ):
                jsl = slice(j * BT, (j + 1) * BT)
                nc.vector.tensor_scalar(
                    out=zT[:, jsl], in0=h2T[:, jsl], scalar1=mu2[:],
                    scalar2=inv2[:], op0=mybir.AluOpType.subtract,
                    op1=mybir.AluOpType.mult,
                )
                nc.vector.tensor_scalar(
                    out=zT[:, jsl], in0=zT[:, jsl], scalar1=1.0, scalar2=-1.0,
                    op0=mybir.AluOpType.min, op1=mybir.AluOpType.max,
                )

            # ---- phase F: fc3 + bias, staged output ----
            for m in range(B_CORE // 128):
                o_psum = psum_o.tile([128, D_OUT], f32, tag="o")
                nc.tensor.matmul(
                    o_psum[:],
                    zT[:, m * 128 : (m + 1) * 128],
                    w3t_t[:],
                    start=True,
                    stop=True,
                )
                nc.vector.tensor_tensor(
                    out=out_sb[:, m, :], in0=o_psum[:], in1=b3bc[:],
                    op=mybir.AluOpType.add,
                )

            # ---- phase G: single output DMA ----
            nc.sync.dma_start(
                out_d.ap().rearrange("(m p) c -> p m c", p=128), out_sb[:]
            )

    nc.compile()
    return nc


_CACHE = {}


def _get_nc():
    if "nc" not in _CACHE:
        _CACHE["nc"] = build()
    return _CACHE["nc"]


def _prep_in_maps(x, w1, b1, w2, b2, w3, b3):
    # b1/b2 cancel inside the batchnorms (see module docstring); only their
    # presence in the reference graph matters, not their values.
    del b1, b2
    w1sT = np.sign(w1).T.astype(np.float32)          # [784, 64]
    w1sT_pad = np.zeros((NK * 128, H), np.float32)
    w1sT_pad[:D_IN] = w1sT
    w2sT = np.sign(w2).T.astype(np.float32)          # [64, 64]
    w3T = np.ascontiguousarray(w3.T.astype(np.float32))  # [64, 10]
    eye = np.eye(128, dtype=np.float32)
    shared = {
        "w1r": w1sT_pad,
        "w1b": w1sT_pad.astype(ml_dtypes.bfloat16),
        "w2s": w2sT.astype(ml_dtypes.bfloat16),
        "w3t": w3T,
        "b3": np.ascontiguousarray(b3.astype(np.float32)).reshape(1, D_OUT),
        "eye": eye,
    }
    x = np.ascontiguousarray(x.astype(np.float32))
    return [
        {"x": x[i * B_CORE : (i + 1) * B_CORE], **shared}
        for i in range(N_CORES)
    ]


def run(in_maps, **kwargs):
    from concourse.bass_utils import run_bass_kernel_spmd

    return run_bass_kernel_spmd(
        _get_nc(), in_maps, core_ids=list(range(N_CORES)), **kwargs
    )


def kernel(x, w1, b1, w2, b2, w3, b3):
    in_maps = _prep_in_maps(x, w1, b1, w2, b2, w3, b3)
    res = run(in_maps)
    return np.concatenate([r["out"] for r in res.results], axis=0)


# revision 9
# speedup vs baseline: 1.2359x; 1.2359x over previous
"""Trainium2 Bass kernel for nn_Net_17179869915 (binarized dense MLP).

Network (reference semantics, B = 32768):
    h1 = x @ sign(w1).T + b1                      # [B, 64]
    s  = sign(h1 - mean(h1))                      # bn1 scale/clip are sign-invariant
    h2 = s @ sign(w2).T                           # b2 cancels inside bn2
    z  = clip((h2 - mean(h2)) * rsqrt(var(h2) + 1e-5), -1, 1)
    out = z @ w3.T + b3                           # [B, 10]

Data-parallel over 8 NeuronCores (4096 rows each); BN statistics are exact
(global) via two tiny AllReduces.

fc1 precision trick: fp32 matmul on the PE costs 4 cycles/row, but fp32r
(E8M11) runs at 1 cycle/row for free dim >= 256. We transpose x on the PE in
fp32, round to fp32r (DVE copy), compute the bf16 residual x - fp32r(x), and
accumulate  x@W = fp32r(x)@W + residual@W  in PSUM. Residual magnitude is
~2^-12 of x, stored in bf16, so the combined error is ~2^-21 per element —
below fp32 accumulation noise. Validated on HW: 1.6e-7 relative.
"""

import numpy as np
import ml_dtypes

import concourse.bass as bass
import concourse.tile as tile
from concourse import bacc, mybir

f32 = mybir.dt.float32
f32r = mybir.dt.float32r
bf16 = mybir.dt.bfloat16

B_TOTAL = 32768
N_CORES = 8
B_CORE = B_TOTAL // N_CORES      # 4096
BT = 512                         # batch tile (free dim of fc1 matmuls)
NJ = B_CORE // BT                # 8 batch tiles per core
NI = BT // 128                   # 4 natural x sub-tiles per batch tile
D_IN = 784
NK = 7                           # ceil(784 / 128) feature chunks
K_LAST = D_IN - 6 * 128          # 16
H = 64
D_OUT = 10
BN_EPS = 1e-5


def build(warmup=True, act_cast=True, ttr=False):
    nc = bacc.Bacc("TRN2", target_bir_lowering=False)

    x_d = nc.dram_tensor("x", [B_CORE, D_IN], f32, kind="ExternalInput")
    w1r_d = nc.dram_tensor("w1r", [NK * 128, H], f32r, kind="ExternalInput")
    w1b_d = nc.dram_tensor("w1b", [NK * 128, H], bf16, kind="ExternalInput")
    w2s_d = nc.dram_tensor("w2s", [H, H], bf16, kind="ExternalInput")
    w3t_d = nc.dram_tensor("w3t", [H, D_OUT], f32, kind="ExternalInput")
    b3_d = nc.dram_tensor("b3", [1, D_OUT], f32, kind="ExternalInput")
    eye_d = nc.dram_tensor("eye", [128, 128], f32, kind="ExternalInput")
    out_d = nc.dram_tensor("out", [B_CORE, D_OUT], f32, kind="ExternalOutput")

    with tile.TileContext(nc) as tc:
        with (
            tc.tile_pool(name="wpool", bufs=1) as wpool,
            tc.tile_pool(name="xin", bufs=2) as xin_pool,
            tc.tile_pool(name="xsplit", bufs=2) as xsplit_pool,
            tc.tile_pool(name="persist", bufs=1) as persist,
            tc.tile_pool(name="small", bufs=1) as small,
            tc.tile_pool(name="psum_xt", bufs=2, space="PSUM") as psum_xt,
            tc.tile_pool(name="psum_h", bufs=2, space="PSUM") as psum_h,
            tc.tile_pool(name="psum_o", bufs=2, space="PSUM") as psum_o,
            tc.tile_pool(name="dram", bufs=1, space="DRAM") as dram,
        ):
            # ---- weights / constants ----
            w1r_t = wpool.tile([128, NK, H], f32r)
            w1b_t = wpool.tile([128, NK, H], bf16)
            w2s_t = wpool.tile([H, H], bf16)
            w3t_t = wpool.tile([H, D_OUT], f32)
            eye_t = wpool.tile([128, 128], f32)
            b3row = wpool.tile([1, D_OUT], f32)
            b3bc = wpool.tile([128, D_OUT], f32)
            nc.sync.dma_start(w1r_t[:], w1r_d.ap().rearrange("(c p) h -> p c h", p=128))
            nc.sync.dma_start(w1b_t[:], w1b_d.ap().rearrange("(c p) h -> p c h", p=128))
            nc.sync.dma_start(w2s_t[:], w2s_d[:])
            nc.sync.dma_start(w3t_t[:], w3t_d[:])
            nc.sync.dma_start(b3row[:], b3_d[:])
            nc.sync.dma_start(eye_t[:], eye_d[:])
            nc.gpsimd.partition_broadcast(b3bc[:], b3row[:])

            # Warm-up collective: absorbs CC-core cold start and the
            # per-core launch stagger while phase A computes. Result unused.
            if warmup:
              wu_in = dram.tile([8], f32)
              wu_out = dram.tile([8], f32, addr_space="Shared")
              wu_sb = small.tile([1, 8], f32)
              nc.vector.memset(wu_sb[:], 0.0)
              nc.sync.dma_start(wu_in[:], wu_sb[:])
              nc.gpsimd.collective_compute(
                  "AllReduce",
                  mybir.AluOpType.add,
                  replica_groups=[list(range(N_CORES))],
                  ins=[wu_in.opt()],
                  outs=[wu_out.opt()],
              )


            # ---- persistent activations (feature-major) ----
            h1T = persist.tile([H, B_CORE], f32)
            sT = persist.tile([H, B_CORE], bf16)
            h2T = persist.tile([H, B_CORE], f32)
            zT = persist.tile([H, B_CORE], f32)
            out_sb = persist.tile([128, B_CORE // 128, D_OUT], f32)

            h1sum = small.tile([H, NJ], f32)
            h2sum = small.tile([H, NJ], f32)
            h2ss = small.tile([H, NJ], f32)
            sq_scrap = small.tile([H, BT], f32)

            # ---- phase A: transpose x, split fp32r+residual, fc1 ----
            for j in range(NJ):
                x_nat = xin_pool.tile([128, NI, D_IN], f32)
                nc.sync.dma_start(
                    x_nat[:],
                    x_d.ap()[j * BT : (j + 1) * BT, :].rearrange(
                        "(i p) f -> p i f", p=128
                    ),
                )
                xr_t = xsplit_pool.tile([128, NK, BT], f32r, tag="xr")
                xres_t = xsplit_pool.tile([128, NK, BT], bf16, tag="xres")
                for k in range(NK):
                    kp = K_LAST if k == NK - 1 else 128
                    xt_psum = psum_xt.tile([128, BT], f32, tag="xt")
                    for i in range(NI):
                        nc.tensor.transpose(
                            xt_psum[0:kp, i * 128 : (i + 1) * 128],
                            x_nat[:, i, k * 128 : k * 128 + kp],
                            eye_t[:],
                        )
                    if act_cast:
                        nc.scalar.activation(
                            xr_t[0:kp, k, :], xt_psum[0:kp, :],
                            mybir.ActivationFunctionType.Copy,
                        )
                    else:
                        nc.vector.tensor_copy(xr_t[0:kp, k, :], xt_psum[0:kp, :])
                    nc.vector.tensor_tensor(
                        out=xres_t[0:kp, k, :],
                        in0=xt_psum[0:kp, :],
                        in1=xr_t[0:kp, k, :].bitcast(f32),
                        op=mybir.AluOpType.subtract,
                    )
                h1_psum = psum_h.tile([H, BT], f32, tag="h")
                for k in range(NK):
                    kp = K_LAST if k == NK - 1 else 128
                    nc.tensor.matmul(
                        h1_psum[:],
                        w1r_t[0:kp, k, :],
                        xr_t[0:kp, k, :],
                        start=(k == 0),
                        stop=False,
                    )
                for k in range(NK):
                    kp = K_LAST if k == NK - 1 else 128
                    nc.tensor.matmul(
                        h1_psum[:],
                        w1b_t[0:kp, k, :],
                        xres_t[0:kp, k, :],
                        start=False,
                        stop=(k == NK - 1),
                    )
                nc.scalar.activation(
                    h1T[:, j * BT : (j + 1) * BT],
                    h1_psum[:],
                    mybir.ActivationFunctionType.Copy,
                    accum_out=h1sum[:, j : j + 1],
                )

            # ---- phase B: global mean of h1 ----
            hsumL = small.tile([H, 1], f32)
            nc.vector.tensor_reduce(
                hsumL[:], h1sum[:], mybir.AxisListType.X, mybir.AluOpType.add
            )
            cc1_in = dram.tile([H], f32)
            cc1_out = dram.tile([H], f32, addr_space="Shared")
            nc.sync.dma_start(cc1_in[:], hsumL[:])
            nc.gpsimd.collective_compute(
                "AllReduce",
                mybir.AluOpType.add,
                replica_groups=[list(range(N_CORES))],
                ins=[cc1_in.opt()],
                outs=[cc1_out.opt()],
            )
            hsumG = small.tile([H, 1], f32)
            nc.sync.dma_start(hsumG[:], cc1_out[:])
            negmu1 = small.tile([H, 1], f32)
            nc.vector.tensor_scalar(
                out=negmu1[:], in0=hsumG[:], scalar1=-1.0 / B_TOTAL, scalar2=None,
                op0=mybir.AluOpType.mult,
            )

            # ---- phase C: sign, fc2, h2 stats ----
            for j in range(NJ):
                jsl = slice(j * BT, (j + 1) * BT)
                nc.scalar.activation(
                    sT[:, jsl], h1T[:, jsl],
                    mybir.ActivationFunctionType.Sign, bias=negmu1[:],
                )
                h2_psum = psum_h.tile([H, BT], f32, tag="h")
                nc.tensor.matmul(
                    h2_psum[:], w2s_t[:], sT[:, jsl], start=True, stop=True
                )
                nc.scalar.activation(
                    h2T[:, jsl], h2_psum[:],
                    mybir.ActivationFunctionType.Copy,
                    accum_out=h2sum[:, j : j + 1],
                )
                if ttr:
                    nc.vector.tensor_tensor_reduce(
                        out=sq_scrap[:],
                        in0=h2T[:, jsl],
                        in1=h2T[:, jsl],
                        scale=1.0,
                        scalar=0.0,
                        op0=mybir.AluOpType.mult,
                        op1=mybir.AluOpType.add,
                        accum_out=h2ss[:, j : j + 1],
                    )
                else:
                    nc.scalar.activation(
                        sq_scrap[:], h2_psum[:],
                        mybir.ActivationFunctionType.Square,
                        accum_out=h2ss[:, j : j + 1],
                    )

            # ---- phase D: global bn2 stats ----
            s2L = small.tile([H, 1], f32)
            ssL = small.tile([H, 1], f32)
            nc.vector.tensor_reduce(
                s2L[:], h2sum[:], mybir.AxisListType.X, mybir.AluOpType.add
            )
            nc.vector.tensor_reduce(
                ssL[:], h2ss[:], mybir.AxisListType.X, mybir.AluOpType.add
            )
            cc2_in = dram.tile([2 * H], f32)
            cc2_out = dram.tile([2 * H], f32, addr_space="Shared")
            nc.sync.dma_start(cc2_in[0:H], s2L[:])
            nc.sync.dma_start(cc2_in[H : 2 * H], ssL[:])
            nc.gpsimd.collective_compute(
                "AllReduce",
                mybir.AluOpType.add,
                replica_groups=[list(range(N_CORES))],
                ins=[cc2_in.opt()],
                outs=[cc2_out.opt()],
            )
            s2G = small.tile([H, 1], f32)
            ssG = small.tile([H, 1], f32)
            nc.sync.dma_start(s2G[:], cc2_out[0:H])
            nc.sync.dma_start(ssG[:], cc2_out[H : 2 * H])

            mu2 = small.tile([H, 1], f32)
            e2 = small.tile([H, 1], f32)
            mu2sq = small.tile([H, 1], f32)
            vareps = small.tile([H, 1], f32)
            rec = small.tile([H, 1], f32)
            inv2 = small.tile([H, 1], f32)
            nc.vector.tensor_scalar(
                out=mu2[:], in0=s2G[:], scalar1=1.0 / B_TOTAL, scalar2=None,
                op0=mybir.AluOpType.mult,
            )
            nc.vector.tensor_scalar(
                out=e2[:], in0=ssG[:], scalar1=1.0 / B_TOTAL, scalar2=None,
                op0=mybir.AluOpType.mult,
            )
            nc.vector.tensor_tensor(
                out=mu2sq[:], in0=mu2[:], in1=mu2[:], op=mybir.AluOpType.mult
            )
            nc.vector.tensor_tensor(
                out=vareps[:], in0=e2[:], in1=mu2sq[:], op=mybir.AluOpType.subtract
            )
            nc.vector.tensor_scalar(
                out=vareps[:], in0=vareps[:], scalar1=BN_EPS, scalar2=None,
                op0=mybir.AluOpType.add,
            )
            nc.vector.reciprocal(rec[:], vareps[:])
            nc.scalar.activation(
                inv2[:], rec[:], mybir.ActivationFunctionType.Sqrt
            )

            # ---- phase E: z = clip((h2 - mu2) * inv2) ----
            for j in range(NJ):
                jsl = slice(j * BT, (j + 1) * BT)
                nc.vector.tensor_scalar(
                    out=zT[:, jsl], in0=h2T[:, jsl], scalar1=mu2[:],
                    scalar2=inv2[:], op0=mybir.AluOpType.subtract,
                    op1=mybir.AluOpType.mult,
                )
                nc.vector.tensor_scalar(
                    out=zT[:, jsl], in0=zT[:, jsl], scalar1=1.0, scalar2=-1.0,
                    op0=mybir.AluOpType.min, op1=mybir.AluOpType.max,
                )

            # ---- phase F: fc3 + bias, staged output ----
            for m in range(B_CORE // 128):
                o_psum = psum_o.tile([128, D_OUT], f32, tag="o")
                nc.tensor.matmul(
                    o_psum[:],
                    zT[:, m * 128 : (m + 1) * 128],
                    w3t_t[:],
                    start=True,
                    stop=True,
                )
                nc.vector.tensor_tensor(
                    out=out_sb[:, m, :], in0=o_psum[:], in1=b3bc[:],
                    op=mybir.AluOpType.add,
                )

            # ---- phase G: single output DMA ----
            nc.sync.dma_start(
                out_d.ap().rearrange("(m p) c -> p m c", p=128), out_sb[:]
            )

    nc.compile()
    return nc


_CACHE = {}


def _get_nc():
    if "nc" not in _CACHE:
        _CACHE["nc"] = build()
    return _CACHE["nc"]


def _prep_in_maps(x, w1, b1, w2, b2, w3, b3):
    # b1/b2 cancel inside the batchnorms (see module docstring); only their
    # presence in the reference graph matters, not their values.
    del b1, b2
    w1sT = np.sign(w1).T.astype(np.float32)          # [784, 64]
    w1sT_pad = np.zeros((NK * 128, H), np.float32)
    w1sT_pad[:D_IN] = w1sT
    w2sT = np.sign(w2).T.astype(np.float32)          # [64, 64]
    w3T = np.ascontiguousarray(w3.T.astype(np.float32))  # [64, 10]
    eye = np.eye(128, dtype=np.float32)
    shared = {
        "w1r": w1sT_pad,
        "w1b": w1sT_pad.astype(ml_dtypes.bfloat16),
        "w2s": w2sT.astype(ml_dtypes.bfloat16),
        "w3t": w3T,
        "b3": np.ascontiguousarray(b3.astype(np.float32)).reshape(1, D_OUT),
        "eye": eye,
    }
    x = np.ascontiguousarray(x.astype(np.float32))
    return [
        {"x": x[i * B_CORE : (i + 1) * B_CORE], **shared}
        for i in range(N_CORES)
    ]


def run(in_maps, **kwargs):
    from concourse.bass_utils import run_bass_kernel_spmd

    return run_bass_kernel_spmd(
        _get_nc(), in_maps, core_ids=list(range(N_CORES)), **kwargs
    )


def kernel(x, w1, b1, w2, b2, w3, b3):
    in_maps = _prep_in_maps(x, w1, b1, w2, b2, w3, b3)
    res = run(in_maps)
    return np.concatenate([r["out"] for r in res.results], axis=0)


# revision 14
# speedup vs baseline: 1.2865x; 1.0410x over previous
"""Trainium2 Bass kernel for nn_Net_17179869915 (binarized dense MLP).

Network (reference semantics, B = 32768):
    h1 = x @ sign(w1).T + b1                      # [B, 64]
    s  = sign(h1 - mean(h1))                      # bn1 scale/clip are sign-invariant
    h2 = s @ sign(w2).T                           # b2 cancels inside bn2
    z  = clip((h2 - mean(h2)) * rsqrt(var(h2) + 1e-5), -1, 1)
    out = z @ w3.T + b3                           # [B, 10]

Data-parallel over 8 NeuronCores (4096 rows each); BN statistics are exact
(global) via two tiny AllReduces.

fc1 precision: fp32 matmul on the PE is 4 cycles/row, but fp32r (E8M11)
runs at 1 cycle/row for free dim >= 256. x is transposed on the PE in fp32,
rounded to fp32r (scalar-engine copy), and the bf16 residual x - fp32r(x)
is accumulated in a second matmul pass:  x@W = fp32r(x)@W + residual@W.
Combined error ~2^-21 per element — below fp32 accumulation noise.

bn1 mean: mean(h1) = mean_b(x) @ sign(w1).T (+b1, which cancels). Column
sums of x are computed with ones-vector fp32r matmuls on the raw x tiles as
they stream in (fp32r read truncates x to 20 bits; the resulting mean error
~2e-5 is ~fp32-tie level), so the first AllReduce launches as soon as the
input DMA finishes and overlaps the fc1 matmul backlog.
"""

import numpy as np
import ml_dtypes

import concourse.bass as bass
import concourse.tile as tile
from concourse import bacc, mybir

f32 = mybir.dt.float32
f32r = mybir.dt.float32r
bf16 = mybir.dt.bfloat16

B_TOTAL = 32768
N_CORES = 8
B_CORE = B_TOTAL // N_CORES      # 4096
BT = 512                         # batch tile (free dim of fc1 matmuls)
NJ = B_CORE // BT                # 8 batch tiles per core
NI = BT // 128                   # 4 natural x sub-tiles per batch tile
D_IN = 784
NK = 7                           # ceil(784 / 128) feature chunks
K_LAST = D_IN - 6 * 128          # 16
H = 64
D_OUT = 10
BN_EPS = 1e-5


def build(warmup=True, xbar_mean=True):
    nc = bacc.Bacc("TRN2", target_bir_lowering=False)

    x_d = nc.dram_tensor("x", [B_CORE, D_IN], f32r, kind="ExternalInput")
    w1r_d = nc.dram_tensor("w1r", [NK * 128, H], f32r, kind="ExternalInput")
    w1b_d = nc.dram_tensor("w1b", [NK * 128, H], bf16, kind="ExternalInput")
    w2s_d = nc.dram_tensor("w2s", [H, H], bf16, kind="ExternalInput")
    w3t_d = nc.dram_tensor("w3t", [H, D_OUT], f32, kind="ExternalInput")
    b3_d = nc.dram_tensor("b3", [1, D_OUT], f32, kind="ExternalInput")
    ones_d = nc.dram_tensor("ones1", [128, 2], f32r, kind="ExternalInput")
    eye_d = nc.dram_tensor("eye", [128, 128], f32, kind="ExternalInput")
    out_d = nc.dram_tensor("out", [B_CORE, D_OUT], f32, kind="ExternalOutput")

    with tile.TileContext(nc) as tc:
        with (
            tc.tile_pool(name="wpool", bufs=1) as wpool,
            tc.tile_pool(name="xin", bufs=2) as xin_pool,
            tc.tile_pool(name="xsplit", bufs=2) as xsplit_pool,
            tc.tile_pool(name="persist", bufs=1) as persist,
            tc.tile_pool(name="small", bufs=1) as small,
            tc.tile_pool(name="psum_xt", bufs=3, space="PSUM") as psum_xt,
            tc.tile_pool(name="psum_h", bufs=2, space="PSUM") as psum_h,
            tc.tile_pool(name="psum_o", bufs=1, space="PSUM") as psum_o,
            tc.tile_pool(name="psum_xs", bufs=1, space="PSUM") as psum_xs,
            tc.tile_pool(name="dram", bufs=1, space="DRAM") as dram,
        ):
            # ---- weights / constants ----
            w1r_t = wpool.tile([128, NK, H], f32r)
            w1b_t = wpool.tile([128, NK, H], bf16)
            w2s_t = wpool.tile([H, H], bf16)
            w3t_t = wpool.tile([H, D_OUT], f32)
            eye_t = wpool.tile([128, 128], f32)
            ones_t = wpool.tile([128, 2], f32r)
            b3row = wpool.tile([1, D_OUT], f32)
            b3bc = wpool.tile([128, D_OUT], f32)
            nc.sync.dma_start(w1r_t[:], w1r_d.ap().rearrange("(c p) h -> p c h", p=128))
            nc.sync.dma_start(w1b_t[:], w1b_d.ap().rearrange("(c p) h -> p c h", p=128))
            nc.sync.dma_start(w2s_t[:], w2s_d[:])
            nc.sync.dma_start(w3t_t[:], w3t_d[:])
            nc.sync.dma_start(b3row[:], b3_d[:])
            nc.sync.dma_start(ones_t[:], ones_d[:])
            nc.sync.dma_start(eye_t[:], eye_d[:])
            nc.gpsimd.partition_broadcast(b3bc[:], b3row[:])

            # Warm-up collective: absorbs CC cold start + launch stagger.
            if warmup:
                wu_in = dram.tile([8], f32)
                wu_out = dram.tile([8], f32, addr_space="Shared")
                wu_sb = small.tile([1, 8], f32)
                nc.vector.memset(wu_sb[:], 0.0)
                nc.sync.dma_start(wu_in[:], wu_sb[:])
                nc.gpsimd.collective_compute(
                    "AllReduce",
                    mybir.AluOpType.add,
                    replica_groups=[list(range(N_CORES))],
                    ins=[wu_in.opt()],
                    outs=[wu_out.opt()],
                )

            # ---- persistent activations (feature-major) ----
            h1T = persist.tile([H, B_CORE], f32)
            sT = persist.tile([H, B_CORE], bf16)
            h2T = persist.tile([H, B_CORE], f32)
            out_sb = persist.tile([128, B_CORE // 128, D_OUT], f32)

            h1sum = small.tile([H, NJ], f32)
            h2sum = small.tile([H, NJ], f32)
            h2ss = small.tile([H, NJ], f32)
            sq_scrap = small.tile([H, BT], f32)

            if xbar_mean:
                xsum_psum = psum_xs.tile([1, D_IN], f32)

            # ---- phase A: transpose x, split fp32r+residual, fc1 ----
            for j in range(NJ):
                x_nat = xin_pool.tile([128, NI, D_IN], f32r)
                nc.sync.dma_start(
                    x_nat[:],
                    x_d.ap()[j * BT : (j + 1) * BT, :].rearrange(
                        "(i p) f -> p i f", p=128
                    ),
                )
                if xbar_mean:
                    # column sums of x (fp32r-truncated) for the bn1 mean —
                    # reads raw x tiles, so these finish with the input DMA.
                    for i in range(NI):
                        nc.tensor.matmul(
                            xsum_psum[0:1, 0:512],
                            ones_t[:, 0:1],
                            x_nat[:, i, 0:512],
                            start=(j == 0 and i == 0),
                            stop=(j == NJ - 1 and i == NI - 1),
                        )
                        nc.tensor.matmul(
                            xsum_psum[0:1, 512:D_IN],
                            ones_t[:, 0:1],
                            x_nat[:, i, 512:D_IN],
                            start=(j == 0 and i == 0),
                            stop=(j == NJ - 1 and i == NI - 1),
                        )
                xr_t = xsplit_pool.tile([128, NK, BT], f32r, tag="xr")
                xres_t = xsplit_pool.tile([128, NK, BT], bf16, tag="xres")
                for k in range(NK):
                    kp = K_LAST if k == NK - 1 else 128
                    xt_psum = psum_xt.tile([128, BT], f32, tag="xt")
                    for i in range(NI):
                        nc.tensor.transpose(
                            xt_psum[0:kp, i * 128 : (i + 1) * 128],
                            x_nat[:, i, k * 128 : k * 128 + kp].bitcast(f32),
                            eye_t[:],
                        )
                    nc.scalar.activation(
                        xr_t[0:kp, k, :], xt_psum[0:kp, :],
                        mybir.ActivationFunctionType.Copy,
                    )
                    nc.vector.tensor_tensor(
                        out=xres_t[0:kp, k, :],
                        in0=xt_psum[0:kp, :],
                        in1=xr_t[0:kp, k, :].bitcast(f32),
                        op=mybir.AluOpType.subtract,
                    )
                h1_psum = psum_h.tile([H, BT], f32, tag="h")
                for k in range(NK):
                    kp = K_LAST if k == NK - 1 else 128
                    nc.tensor.matmul(
                        h1_psum[:],
                        w1r_t[0:kp, k, :],
                        xr_t[0:kp, k, :],
                        start=(k == 0),
                        stop=False,
                    )
                for k in range(NK):
                    kp = K_LAST if k == NK - 1 else 128
                    nc.tensor.matmul(
                        h1_psum[:],
                        w1b_t[0:kp, k, :],
                        xres_t[0:kp, k, :],
                        start=False,
                        stop=(k == NK - 1),
                    )
                if xbar_mean:
                    nc.scalar.activation(
                        h1T[:, j * BT : (j + 1) * BT],
                        h1_psum[:],
                        mybir.ActivationFunctionType.Copy,
                    )
                else:
                    nc.scalar.activation(
                        h1T[:, j * BT : (j + 1) * BT],
                        h1_psum[:],
                        mybir.ActivationFunctionType.Copy,
                        accum_out=h1sum[:, j : j + 1],
                    )

            # ---- phase B: global bn1 mean ----
            negmu1 = small.tile([H, 1], f32)
            if xbar_mean:
                xsb = small.tile([1, NK * 128], f32)
                nc.vector.memset(xsb[:, D_IN:], 0.0)
                nc.scalar.activation(
                    xsb[:, 0:D_IN], xsum_psum[:], mybir.ActivationFunctionType.Copy
                )
                cc1_in = dram.tile([NK * 128], f32)
                cc1_out = dram.tile([NK * 128], f32, addr_space="Shared")
                nc.sync.dma_start(cc1_in[:], xsb[:])
                nc.gpsimd.collective_compute(
                    "AllReduce",
                    mybir.AluOpType.add,
                    replica_groups=[list(range(N_CORES))],
                    ins=[cc1_in.opt()],
                    outs=[cc1_out.opt()],
                )
                # scatter the 784 summed columns back as [128, 7]
                xbarG = small.tile([128, NK], f32)
                nc.sync.dma_start(
                    xbarG[:],
                    cc1_out[:].rearrange("(c p) -> p c", p=128),
                )
                mu1_psum = psum_o.tile([H, 1], f32, tag="o")
                for k in range(NK):
                    kp = K_LAST if k == NK - 1 else 128
                    nc.tensor.matmul(
                        mu1_psum[:],
                        w1r_t[0:kp, k, :].bitcast(f32),
                        xbarG[0:kp, k : k + 1],
                        start=(k == 0),
                        stop=(k == NK - 1),
                    )
                nc.scalar.activation(
                    negmu1[:], mu1_psum[:],
                    mybir.ActivationFunctionType.Copy,
                    scale=-1.0 / B_TOTAL,
                )
            else:
                hsumL = small.tile([H, 1], f32)
                nc.vector.tensor_reduce(
                    hsumL[:], h1sum[:], mybir.AxisListType.X, mybir.AluOpType.add
                )
                cc1_in = dram.tile([H], f32)
                cc1_out = dram.tile([H], f32, addr_space="Shared")
                nc.sync.dma_start(cc1_in[:], hsumL[:])
                nc.gpsimd.collective_compute(
                    "AllReduce",
                    mybir.AluOpType.add,
                    replica_groups=[list(range(N_CORES))],
                    ins=[cc1_in.opt()],
                    outs=[cc1_out.opt()],
                )
                hsumG = small.tile([H, 1], f32)
                nc.sync.dma_start(hsumG[:], cc1_out[:])
                nc.vector.tensor_scalar(
                    out=negmu1[:], in0=hsumG[:], scalar1=-1.0 / B_TOTAL,
                    scalar2=None, op0=mybir.AluOpType.mult,
                )

            # ---- phase C: sign, fc2, h2 stats ----
            for j in range(NJ):
                jsl = slice(j * BT, (j + 1) * BT)
                nc.scalar.activation(
                    sT[:, jsl], h1T[:, jsl],
                    mybir.ActivationFunctionType.Sign, bias=negmu1[:],
                )
                h2_psum = psum_h.tile([H, BT], f32, tag="h")
                nc.tensor.matmul(
                    h2_psum[:], w2s_t[:], sT[:, jsl], start=True, stop=True
                )
                nc.vector.tensor_scalar(
                    out=h2T[:, jsl], in0=h2_psum[:], scalar1=0.0, scalar2=0.0,
                    op0=mybir.AluOpType.add, op1=mybir.AluOpType.add,
                    accum_out=h2sum[:, j : j + 1],
                )
                nc.scalar.activation(
                    sq_scrap[:], h2_psum[:],
                    mybir.ActivationFunctionType.Square,
                    accum_out=h2ss[:, j : j + 1],
                )

            # ---- phase D: global bn2 stats ----
            s2L = small.tile([H, 1], f32)
            ssL = small.tile([H, 1], f32)
            nc.vector.tensor_reduce(
                s2L[:], h2sum[:], mybir.AxisListType.X, mybir.AluOpType.add
            )
            nc.vector.tensor_reduce(
                ssL[:], h2ss[:], mybir.AxisListType.X, mybir.AluOpType.add
            )
            cc2_in = dram.tile([2 * H], f32)
            cc2_out = dram.tile([2 * H], f32, addr_space="Shared")
            nc.sync.dma_start(cc2_in[0:H], s2L[:])
            nc.sync.dma_start(cc2_in[H : 2 * H], ssL[:])
            nc.gpsimd.collective_compute(
                "AllReduce",
                mybir.AluOpType.add,
                replica_groups=[list(range(N_CORES))],
                ins=[cc2_in.opt()],
                outs=[cc2_out.opt()],
            )
            s2G = small.tile([H, 1], f32)
            ssG = small.tile([H, 1], f32)
            nc.sync.dma_start(s2G[:], cc2_out[0:H])
            nc.sync.dma_start(ssG[:], cc2_out[H : 2 * H])

            mu2 = small.tile([H, 1], f32)
            e2 = small.tile([H, 1], f32)
            mu2sq = small.tile([H, 1], f32)
            vareps = small.tile([H, 1], f32)
            rec = small.tile([H, 1], f32)
            inv2 = small.tile([H, 1], f32)
            nc.vector.tensor_scalar(
                out=mu2[:], in0=s2G[:], scalar1=1.0 / B_TOTAL, scalar2=None,
                op0=mybir.AluOpType.mult,
            )
            nc.vector.tensor_scalar(
                out=e2[:], in0=ssG[:], scalar1=1.0 / B_TOTAL, scalar2=None,
                op0=mybir.AluOpType.mult,
            )
            nc.vector.tensor_tensor(
                out=mu2sq[:], in0=mu2[:], in1=mu2[:], op=mybir.AluOpType.mult
            )
            nc.vector.tensor_tensor(
                out=vareps[:], in0=e2[:], in1=mu2sq[:], op=mybir.AluOpType.subtract
            )
            nc.vector.tensor_scalar(
                out=vareps[:], in0=vareps[:], scalar1=BN_EPS, scalar2=None,
                op0=mybir.AluOpType.add,
            )
            nc.vector.reciprocal(rec[:], vareps[:])
            nc.scalar.activation(
                inv2[:], rec[:], mybir.ActivationFunctionType.Sqrt
            )

            # ---- phase E: z = clip((h2 - mu2) * inv2), in place ----
            for j in range(NJ):
                jsl = slice(j * BT, (j + 1) * BT)
                nc.vector.tensor_scalar(
                    out=h2T[:, jsl], in0=h2T[:, jsl], scalar1=mu2[:],
                    scalar2=inv2[:], op0=mybir.AluOpType.subtract,
                    op1=mybir.AluOpType.mult,
                )
                nc.vector.tensor_scalar(
                    out=h2T[:, jsl], in0=h2T[:, jsl], scalar1=1.0, scalar2=-1.0,
                    op0=mybir.AluOpType.min, op1=mybir.AluOpType.max,
                )

            # ---- phase F: fc3 + bias, staged output ----
            for m in range(B_CORE // 128):
                o_psum = psum_o.tile([128, D_OUT], f32, tag="o")
                nc.tensor.matmul(
                    o_psum[:],
                    h2T[:, m * 128 : (m + 1) * 128],
                    w3t_t[:],
                    start=True,
                    stop=True,
                )
                nc.vector.tensor_tensor(
                    out=out_sb[:, m, :], in0=o_psum[:], in1=b3bc[:],
                    op=mybir.AluOpType.add,
                )

            # ---- phase G: single output DMA ----
            nc.sync.dma_start(
                out_d.ap().rearrange("(m p) c -> p m c", p=128), out_sb[:]
            )

    nc.compile()
    return nc


_CACHE = {}


def _get_nc():
    if "nc" not in _CACHE:
        _CACHE["nc"] = build()
    return _CACHE["nc"]


def _prep_in_maps(x, w1, b1, w2, b2, w3, b3):
    # b1/b2 cancel inside the batchnorms (see module docstring); only their
    # presence in the reference graph matters, not their values.
    del b1, b2
    w1sT = np.sign(w1).T.astype(np.float32)          # [784, 64]
    w1sT_pad = np.zeros((NK * 128, H), np.float32)
    w1sT_pad[:D_IN] = w1sT
    w2sT = np.sign(w2).T.astype(np.float32)          # [64, 64]
    w3T = np.ascontiguousarray(w3.T.astype(np.float32))  # [64, 10]
    eye = np.eye(128, dtype=np.float32)
    shared = {
        "w1r": w1sT_pad,
        "w1b": w1sT_pad.astype(ml_dtypes.bfloat16),
        "w2s": w2sT.astype(ml_dtypes.bfloat16),
        "w3t": w3T,
        "b3": np.ascontiguousarray(b3.astype(np.float32)).reshape(1, D_OUT),
        "ones1": np.ones((128, 2), np.float32),
        "eye": eye,
    }
    x = np.ascontiguousarray(x.astype(np.float32))
    return [
        {"x": x[i * B_CORE : (i + 1) * B_CORE], **shared}
        for i in range(N_CORES)
    ]


def run(in_maps, **kwargs):
    from concourse.bass_utils import run_bass_kernel_spmd

    return run_bass_kernel_spmd(
        _get_nc(), in_maps, core_ids=list(range(N_CORES)), **kwargs
    )


def kernel(x, w1, b1, w2, b2, w3, b3):
    in_maps = _prep_in_maps(x, w1, b1, w2, b2, w3, b3)
    res = run(in_maps)
    return np.concatenate([r["out"] for r in res.results], axis=0)
